# revision 5
# baseline (speedup 1.0000x reference)
"""Llama GQA chunk-attention layer on 8 Trainium2 NeuronCores.

Tensor-parallel over heads: core c computes Q heads [4c, 4c+4), KV head c,
and the partial output (attn_out_c @ Wo[rows of c's heads]); the host sums
the 8 partials and divides by the folded power-of-2 scale.

All four projection GEMMs run as residual-fp8 DoubleRow matmuls at 1.5x
the bf16 rate with ~bf16 accuracy: each operand A is split into
A_hi = e4m3(A) and A_lo = e4m3(A - A_hi); per 128-slab one DR matmul
computes (A_hi+A_lo)^T B_hi (B_hi slab broadcast via stride-0 AP) and per
slab-PAIR one DR matmul adds A_hi^T B_lo, dropping only the lo*lo term
(~0.1%). 32-slab contraction: 48 DR matmuls at 0.5 cyc/col = 24 col-cycles
vs 32 for bf16.

Scale folding (inputs x4, weights x512 to center fp8 exponents):
cos/sin /2048, rowsum 'ones' = 256 (-> aT = 8*attn), host output /4096.

Software-pipelined schedule:
  phase A: projections for batch-0 blocks (block 0 streamed against the
           weight DMA order so PE starts ~3.5us in)
  phase B: batch-1 projections with batch-0 attention units interleaved;
           Wo halves stream into the freed xt double-buffer slots
  phase C: batch-1 attention with output-projection half-tiles as filler,
           then the remaining output projection.

Shapes (hardcoded): B=2, S=2048, HIDDEN=4096, 32 Q heads, 8 KV heads, D=128.
"""

import math
from contextlib import ExitStack

import ml_dtypes
import numpy as np

import concourse.bass as bass
import concourse.mybir as mybir
import concourse.tile as tile
from concourse.bass_utils import run_bass_kernel_spmd

# Problem constants
B, S, HID = 2, 2048, 4096
T = B * S                  # 4096 tokens
NH, NKV, D = 32, 8, 128
NCORES = 8
NH_C = NH // NCORES        # 4 q heads per core
DQ_C = NH_C * D            # 512
ROPE_BASE = 10000.0
SCALE = D ** -0.5

P = 128                    # partitions
TOKBLK = 256               # X^T chunk width for projections
NTOKBLK = T // TOKBLK      # 16
HO = HID // P              # 32 hidden 128-tiles
NTT = T // P               # 32 token 128-tiles
QBLK = 512                 # attention q-block width
NQB = S // QBLK            # 4 q-blocks per batch
KT_PER_B = S // P          # 16 k-tiles per batch
H2 = HID // 2              # 2048

SX = 4.0                   # input scale
SW = 512.0                 # weight scale
ONESV = 256.0              # rowsum scale -> aT = 8*attn
OUT_DIV = 4096.0           # host unscale (8 * 512)

BF16 = mybir.dt.bfloat16
F32 = mybir.dt.float32
F8E5 = mybir.dt.float8e5
F8E4 = mybir.dt.float8e4
DRM = mybir.MatmulPerfMode.DoubleRow


def build_bass():
    nc = bass.Bass()

    xt = nc.dram_tensor("xt", [P, NTOKBLK, HO, 2, TOKBLK], F8E4,
                        kind="ExternalInput")
    wq = nc.dram_tensor("wq", [P, NH_C, HO, 2, P], F8E4,
                        kind="ExternalInput")
    wk = nc.dram_tensor("wk", [P, HO, 2, P], F8E4, kind="ExternalInput")
    wv = nc.dram_tensor("wv", [P, HO, 2, P], F8E4, kind="ExternalInput")
    wo = nc.dram_tensor("wo", [P, NH_C, 2, HID], F8E4,
                        kind="ExternalInput")
    cos = nc.dram_tensor("cos", [D // 2, T], BF16, kind="ExternalInput")
    sin = nc.dram_tensor("sin", [D // 2, T], BF16, kind="ExternalInput")
    maskv = nc.dram_tensor("maskv", [P, P], BF16, kind="ExternalInput")
    ones = nc.dram_tensor("ones", [P, P], BF16, kind="ExternalInput")
    ones8 = nc.dram_tensor("ones8", [P, 2 * P], F8E5, kind="ExternalInput")
    out = nc.dram_tensor("out", [T, HID], BF16, kind="ExternalOutput")

    with tile.TileContext(nc) as tc, ExitStack() as ctx:
        singles = ctx.enter_context(tc.tile_pool(name="singles", bufs=1))
        tmps = ctx.enter_context(tc.tile_pool(name="tmps", bufs=1))
        ptpool = ctx.enter_context(tc.tile_pool(name="ptpool", bufs=5))
        p8pool = ctx.enter_context(tc.tile_pool(name="p8pool", bufs=8))
        rpool = ctx.enter_context(tc.tile_pool(name="rpool", bufs=1))
        opool = ctx.enter_context(tc.tile_pool(name="opool", bufs=3))

        # ---- resident SBUF tensors ----
        wq_sb = singles.tile([P, NH_C, HO, 2, P], F8E4, tag="wq")
        wk_sb = singles.tile([P, HO, 2, P], F8E4, tag="wk")
        wv_sb = singles.tile([P, HO, 2, P], F8E4, tag="wv")
        qT_sb = singles.tile([P, NH_C, T], BF16, tag="qT")
        kT_sb = singles.tile([P, T], BF16, tag="kT")
        v_sb = singles.tile([P, NTT, D], BF16, tag="v")
        aT_sb = singles.tile([P, NH_C, 2, T], F8E4, tag="aT")  # hi/lo
        # cos on partitions [0:64), sin on [64:128)
        cs_sb = singles.tile([P, T], BF16, tag="cossin")
        cos_sb = cs_sb[0:D // 2, :]
        sin_sb = cs_sb[D // 2:P, :]
        maskv_sb = singles.tile([P, P], BF16, tag="maskv")
        ones_sb = singles.tile([P, P], BF16, tag="ones")
        ones8_sb = singles.tile([P, 2, P], F8E5, tag="ones8")
        scratch = singles.tile([1, 4], F32, tag="scratch")

        def rope(dst_lo, dst_hi, src_psum, t0):
            # dst = [src_lo*cos - src_hi*sin ; src_hi*cos + src_lo*sin]
            c_sl = cos_sb[:, t0:t0 + TOKBLK]
            s_sl = sin_sb[:, t0:t0 + TOKBLK]
            h = D // 2
            # 1-element copy absorbs the PE wait so the tensor_tensor ops
            # below carry <=1 sync wait (TT ISA slot limit).
            nc.vector.tensor_copy(scratch[0:1, 0:1], src_psum[0:1, 0:1])
            ta = tmps.tile([h, TOKBLK], F32, tag="ta", name="ta")
            tb = tmps.tile([h, TOKBLK], F32, tag="tb", name="tb")
            nc.vector.tensor_mul(ta, src_psum[0:h, :], c_sl)
            nc.vector.tensor_mul(tb, src_psum[h:P, :], s_sl)
            nc.vector.tensor_sub(dst_lo, ta, tb)
            ta2 = tmps.tile([h, TOKBLK], F32, tag="ta", name="ta2")
            tb2 = tmps.tile([h, TOKBLK], F32, tag="tb", name="tb2")
            nc.vector.tensor_mul(ta2, src_psum[h:P, :], c_sl)
            nc.vector.tensor_mul(tb2, src_psum[0:h, :], s_sl)
            nc.vector.tensor_add(dst_hi, ta2, tb2)

        def xt_tag(tb):
            return "xtA" if tb % 2 == 0 else "xtB"

        def load_xt(tb):
            xt_sb = singles.tile([P, HO, 2, TOKBLK], F8E4, tag=xt_tag(tb),
                                 name=f"xt{tb}")
            for i in range(2):
                nc.sync.dma_start(
                    xt_sb[:, i * 16:(i + 1) * 16, :, :],
                    xt[:, tb, i * 16:(i + 1) * 16, :, :])
            return xt_sb

        # ---- residual-fp8 DR projection groups ----
        def wx_group(ps, w4, xs, ho_range, last):
            """psum += W^T X over ho slabs: lhsT = [W_hi;W_lo] per slab
            (DR1, X_hi broadcast) + [W_hi_k;W_hi_k+1] x [X_lo_k;X_lo_k+1]
            (DR2 per pair). w4: [P, HO, 2, P]-like slice source."""
            ho0, ho1 = ho_range
            n = xs.shape[-1]
            for ho in range(ho0, ho1):
                nc.tensor.matmul(
                    ps, lhsT=w4[:, ho, :, :],
                    rhs=xs[:, ho, 0:1, :].broadcast_to((P, 2, n)),
                    start=(ho == 0), stop=False,
                    perf_mode=DRM, skip_group_check=True)
            for pr in range(ho0 // 2, ho1 // 2):
                nc.tensor.matmul(
                    ps, lhsT=w4[:, 2 * pr:2 * pr + 2, 0, :],
                    rhs=xs[:, 2 * pr:2 * pr + 2, 1, :],
                    start=False, stop=(last and pr == ho1 // 2 - 1),
                    perf_mode=DRM, skip_group_check=True)

        def xv_group(ps, xs, tsl, ho_range, last):
            """psum += X^T Wv over ho slabs (out = [tok, D])."""
            ho0, ho1 = ho_range
            for ho in range(ho0, ho1):
                nc.tensor.matmul(
                    ps, lhsT=xs[:, ho, :, tsl],
                    rhs=wv_sb[:, ho, 0:1, :].broadcast_to((P, 2, P)),
                    start=(ho == 0), stop=False,
                    perf_mode=DRM, skip_group_check=True)
            for pr in range(ho0 // 2, ho1 // 2):
                nc.tensor.matmul(
                    ps, lhsT=xs[:, 2 * pr:2 * pr + 2, 0, tsl],
                    rhs=wv_sb[:, 2 * pr:2 * pr + 2, 1, :],
                    start=False, stop=(last and pr == ho1 // 2 - 1),
                    perf_mode=DRM, skip_group_check=True)

        # ================= PHASE A: batch-0 projections =================
        with tc.tile_pool(name="ppA", bufs=8, space="PSUM") as ppA:
            # --- block 0 streamed against the weight DMA order ---
            xt0_sb = singles.tile([P, HO, 2, TOKBLK], F8E4, tag="xtA",
                                  name="xt0")
            for q in range(4):
                ho0 = q * 8
                if q == 0:
                    nc.sync.dma_start(wk_sb[:, 0:4, :, :], wk[:, 0:4, :, :])
                    nc.sync.dma_start(xt0_sb[:, 0:4, :, :],
                                      xt[:, 0, 0:4, :, :])
                    nc.sync.dma_start(wk_sb[:, 4:8, :, :], wk[:, 4:8, :, :])
                    nc.sync.dma_start(xt0_sb[:, 4:8, :, :],
                                      xt[:, 0, 4:8, :, :])
                else:
                    nc.sync.dma_start(wk_sb[:, ho0:ho0 + 8, :, :],
                                      wk[:, ho0:ho0 + 8, :, :])
                    nc.sync.dma_start(xt0_sb[:, ho0:ho0 + 4, :, :],
                                      xt[:, 0, ho0:ho0 + 4, :, :])
                    nc.sync.dma_start(xt0_sb[:, ho0 + 4:ho0 + 8, :, :],
                                      xt[:, 0, ho0 + 4:ho0 + 8, :, :])
                nc.sync.dma_start(wv_sb[:, ho0:ho0 + 8, :, :],
                                  wv[:, ho0:ho0 + 8, :, :])
                for hq in range(2):
                    nc.sync.dma_start(wq_sb[:, hq, ho0:ho0 + 8, :, :],
                                      wq[:, hq, ho0:ho0 + 8, :, :])
            # cos/sin for batch 0 (needed by ropes to drain PSUM)
            nc.sync.dma_start(cs_sb[0:D // 2, 0:S], cos[:, 0:S])
            nc.sync.dma_start(cs_sb[D // 2:P, 0:S], sin[:, 0:S])
            for hq in (2, 3):
                nc.sync.dma_start(wq_sb[:, hq, 0:16, :, :],
                                  wq[:, hq, 0:16, :, :])
                nc.sync.dma_start(wq_sb[:, hq, 16:32, :, :],
                                  wq[:, hq, 16:32, :, :])
            xt1_sb = load_xt(1)
            nc.sync.dma_start(cs_sb[0:D // 2, S:T], cos[:, S:T])
            nc.sync.dma_start(cs_sb[D // 2:P, S:T], sin[:, S:T])
            nc.sync.dma_start(maskv_sb, maskv[:, :])
            nc.sync.dma_start(ones_sb, ones[:, :])
            nc.sync.dma_start(ones8_sb, ones8.rearrange("p (k m) -> p k m", k=2))
            xt_bufs = {0: xt0_sb, 1: xt1_sb}

            # block-0 K/Q0/Q1/V matmuls accumulated per weight quarter
            psk0 = ppA.tile([P, TOKBLK], F32, tag="st", name="psk0")
            psq0 = [ppA.tile([P, TOKBLK], F32, tag="st", name=f"psq0_{hq}")
                    for hq in range(NH_C)]
            psv0 = [ppA.tile([P, D], F32, tag="st", name=f"psv0_{tt}")
                    for tt in range(2)]
            for q in range(4):
                rng_ = (q * 8, q * 8 + 8)
                wx_group(psk0, wk_sb, xt0_sb, rng_, q == 3)
                for hq in range(2):
                    wx_group(psq0[hq], wq_sb[:, hq, :, :, :], xt0_sb, rng_,
                             q == 3)
                for tt in range(2):
                    xv_group(psv0[tt], xt0_sb,
                             slice(tt * P, (tt + 1) * P), rng_, q == 3)
            rope(kT_sb[0:D // 2, 0:TOKBLK], kT_sb[D // 2:P, 0:TOKBLK],
                 psk0, 0)
            for hq in range(2):
                rope(qT_sb[0:D // 2, hq, 0:TOKBLK],
                     qT_sb[D // 2:P, hq, 0:TOKBLK], psq0[hq], 0)
            for tt in range(2):
                nc.any.tensor_copy(v_sb[:, tt, :], psv0[tt])
            # block-0 Q2/Q3 (their Wq chunks land after the stream)
            for hq in (2, 3):
                wx_group(psq0[hq], wq_sb[:, hq, :, :, :], xt0_sb, (0, HO),
                         True)
                rope(qT_sb[0:D // 2, hq, 0:TOKBLK],
                     qT_sb[D // 2:P, hq, 0:TOKBLK], psq0[hq], 0)
            # safe prefetch: xt(tb+2) aliases the slot block tb just read,
            # so the WAR dep exists only if emitted after tb's matmuls
            xt_bufs[2] = load_xt(2)

            def proj_block(tb, pool):
                t0 = tb * TOKBLK
                xt_sb = xt_bufs[tb]
                psk = pool.tile([P, TOKBLK], F32, tag="st", name=f"psk{tb}")
                wx_group(psk, wk_sb, xt_sb, (0, HO), True)
                rope(kT_sb[0:D // 2, t0:t0 + TOKBLK],
                     kT_sb[D // 2:P, t0:t0 + TOKBLK], psk, t0)
                for hq in range(NH_C):
                    psq = pool.tile([P, TOKBLK], F32, tag="st",
                                    name=f"psq{tb}_{hq}")
                    wx_group(psq, wq_sb[:, hq, :, :, :], xt_sb, (0, HO),
                             True)
                    rope(qT_sb[0:D // 2, hq, t0:t0 + TOKBLK],
                         qT_sb[D // 2:P, hq, t0:t0 + TOKBLK], psq, t0)
                for tt in range(2):
                    g = (t0 // P) + tt
                    psv = pool.tile([P, D], F32, tag="st",
                                    name=f"psv{tb}_{tt}")
                    xv_group(psv, xt_sb, slice(tt * P, (tt + 1) * P),
                             (0, HO), True)
                    nc.any.tensor_copy(v_sb[:, g, :], psv)
                if tb + 2 < NTOKBLK:
                    xt_bufs[tb + 2] = load_xt(tb + 2)

            for tb in range(1, 8):
                proj_block(tb, ppA)

        # ============ PHASE B/C: attention + output projection ============
        with tc.tile_pool(name="ps_st", bufs=3, space="PSUM") as ps_st, \
             tc.tile_pool(name="ps_acc", bufs=2, space="PSUM") as ps_acc, \
             tc.tile_pool(name="ps_sum", bufs=1, space="PSUM") as ps_sum:

            def attn_unit(b, hq, qb, filler=None, copy_eng=None):
                cp = copy_eng or nc.gpsimd
                cp_copy = cp.copy if cp is nc.scalar else cp.tensor_copy
                q0 = b * S + qb * QBLK
                nkt = (qb + 1) * (QBLK // P)
                ps_av = ps_acc.tile([P, QBLK], F32, tag="acc")
                ps_rs = ps_sum.tile([P, QBLK], F32, tag="sum")
                state = {}
                drs = []  # DR rowsums after pair 0 are deferred to the
                          # unit end so their fp8 staging copies get a
                          # whole unit of slack off the PE critical path.
                          # The kt==1 DR keeps its validated in-loop spot
                          # (it carries the full-width start=True).

                def front(kt):
                    # scores + exp (+ mask + fp8 staging) for tile kt
                    j = kt - qb * (QBLK // P)
                    c0 = j * P if j >= 0 else 0
                    gk = b * KT_PER_B + kt
                    pst = ps_st.tile([P, QBLK], F32, tag="st")
                    nc.tensor.matmul(
                        pst[:, c0:],
                        lhsT=kT_sb[:, gk * P:(gk + 1) * P],
                        rhs=qT_sb[:, hq, q0 + c0:q0 + QBLK],
                        start=True, stop=True,
                    )
                    pT = ptpool.tile([P, QBLK], BF16, tag="pt")
                    nc.scalar.activation(
                        pT[:, c0:], pst[:, c0:],
                        mybir.ActivationFunctionType.Exp)
                    if j >= 0:
                        nc.vector.tensor_mul(
                            pT[:, c0:c0 + P], pT[:, c0:c0 + P], maskv_sb)
                        if qb > 0 and j in (0, 1):
                            if j == 0:
                                state["pr8d"] = p8pool.tile(
                                    [P, 2, QBLK], F8E5, tag="p8",
                                    name="pr8d")
                            cp_copy(
                                state["pr8d"][:, j, 2 * P:], pT[:, 2 * P:])
                    else:
                        if kt % 2 == 0:
                            state[("pr8", kt // 2)] = p8pool.tile(
                                [P, 2, QBLK], F8E5, tag="p8", name="pr8")
                        cp_copy(state[("pr8", kt // 2)][:, kt % 2, :], pT)
                    state[kt] = pT

                def back(kt):
                    # rowsum matmuls + AV for tile kt
                    j = kt - qb * (QBLK // P)
                    c0 = j * P if j >= 0 else 0
                    gk = b * KT_PER_B + kt
                    pT = state.pop(kt)
                    if j >= 0:
                        if qb == 0:
                            # short rows: keep denominators fully bf16
                            nc.tensor.matmul(
                                ps_rs[:, c0:], lhsT=ones_sb[:, :],
                                rhs=pT[:, c0:],
                                start=(kt == 0), stop=(kt == nkt - 1),
                                skip_group_check=True,
                            )  # qb0: drs stays empty
                        elif j == 0:
                            nc.tensor.matmul(
                                ps_rs[:, 0:2 * P], lhsT=ones_sb[:, :],
                                rhs=pT[:, 0:2 * P],
                                start=False, stop=False,
                                skip_group_check=True,
                            )
                        elif j == 1:
                            drs.append((state["pr8d"], 2 * P))
                            nc.tensor.matmul(
                                ps_rs[:, P:2 * P], lhsT=ones_sb[:, :],
                                rhs=pT[:, P:2 * P],
                                start=False, stop=False,
                                skip_group_check=True,
                            )
                        elif j == 2:
                            nc.tensor.matmul(
                                ps_rs[:, 2 * P:], lhsT=ones_sb[:, :],
                                rhs=pT[:, 2 * P:],
                                start=False, stop=False,
                                skip_group_check=True,
                            )
                        else:
                            nc.tensor.matmul(
                                ps_rs[:, 3 * P:], lhsT=ones_sb[:, :],
                                rhs=pT[:, 3 * P:],
                                start=False, stop=False,
                                skip_group_check=True,
                            )
                    elif kt == 1:
                        # full-width start=True stays in-loop (validated)
                        nc.tensor.matmul(
                            ps_rs, lhsT=ones8_sb[:, :, :],
                            rhs=state.pop(("pr8", 0))[:, :, :],
                            start=True, stop=False,
                            perf_mode=DRM,
                            skip_group_check=True,
                        )
                    elif kt % 2 == 1:
                        drs.append((state.pop(("pr8", kt // 2)), 0))
                    nc.tensor.matmul(
                        ps_av[:, c0:], lhsT=v_sb[:, gk, :],
                        rhs=pT[:, c0:],
                        start=(kt == 0), stop=(kt == nkt - 1),
                        skip_group_check=True,
                    )

                # depth-2 software pipeline: scores run 2 tiles ahead of
                # their rowsum/AV so the exp/mask latency is hidden; the
                # filler (output-projection work) lands in the warm-up gap.
                front(0)
                if nkt > 1:
                    front(1)
                if filler is not None:
                    filler()
                for kt in range(nkt):
                    if kt + 2 < nkt:
                        front(kt + 2)
                    back(kt)
                for i, (pr8t, lo) in enumerate(drs):
                    nc.tensor.matmul(
                        ps_rs[:, lo:], lhsT=ones8_sb[:, :, :],
                        rhs=pr8t[:, :, lo:],
                        start=False, stop=(i == len(drs) - 1),
                        perf_mode=DRM, skip_group_check=True,
                    )
                # aT = 8*attn split into e4m3 hi + lo (r folds the 1/2048
                # projection scale and the *8: ones = 256)
                r = rpool.tile([P, QBLK], F32, tag="r")
                nc.vector.reciprocal(r, ps_rs)
                t_ = rpool.tile([P, QBLK], F32, tag="t", name="t_")
                nc.vector.tensor_mul(t_, ps_av, r)
                nc.gpsimd.tensor_copy(aT_sb[:, hq, 0, q0:q0 + QBLK], t_)
                nc.gpsimd.tensor_sub(aT_sb[:, hq, 1, q0:q0 + QBLK], t_,
                                     aT_sb[:, hq, 0, q0:q0 + QBLK])

            # output-projection half-tile: rows [tt*128,(tt+1)*128) x
            # cols [half*2048,(half+1)*2048), residual-fp8 DR over 4 hq
            def p3_half(tt, half, pool, wo_half, ptag, tail=False,
                        last=False, pool2=None):
                o_t = opool.tile([P, H2], BF16, tag="ot")
                tsl = slice(tt * P, (tt + 1) * P)

                def pso_mms(c0, c1, name):
                    # in the pure-p3 tail the attention pools are idle:
                    # borrow ps_st/ps_acc banks for a deeper pso rotation
                    if pool2 is not None and hbi < 2:
                        pso = pool2.tile([P, QBLK], F32, tag="st", name=name)
                    elif pool2 is not None and hbi == 2:
                        pso = ps_acc.tile([P, QBLK], F32, tag="acc",
                                          name=name)
                    else:
                        pso = pool.tile([P, QBLK], F32, tag=ptag, name=name)
                    n = c1 - c0
                    for hq in range(NH_C):
                        csl = slice(hbi * QBLK + c0, hbi * QBLK + c1)
                        nc.tensor.matmul(
                            pso[:, 0:n],
                            lhsT=aT_sb[:, hq, :, tsl],
                            rhs=wo_half[:, hq, 0:1, csl]
                                .broadcast_to((P, 2, n)),
                            start=(hq == 0), stop=False,
                            perf_mode=DRM, skip_group_check=True,
                        )
                    for pr in range(2):
                        csl = slice(hbi * QBLK + c0, hbi * QBLK + c1)
                        nc.tensor.matmul(
                            pso[:, 0:n],
                            lhsT=aT_sb[:, 2 * pr:2 * pr + 2, 0, tsl],
                            rhs=wo_half[:, 2 * pr:2 * pr + 2, 1, csl],
                            start=False, stop=(pr == 1),
                            perf_mode=DRM, skip_group_check=True,
                        )
                    return pso

                orows = out[tt * P:(tt + 1) * P, :]
                for hbi in range(4):
                    nm = f"pso{tt}_{half}_{hbi}"
                    dst = o_t[:, hbi * QBLK:(hbi + 1) * QBLK]
                    ocol = half * H2 + hbi * QBLK
                    if last and hbi == 3:
                        # shorten the copy->store->drain tail: one store for
                        # hb0-2 flows during hb3; the final PE output is a
                        # 128-col pso with one small DMA behind it
                        nc.sync.dma_start(orows[:, half * H2:ocol],
                                          o_t[:, 0:3 * QBLK])
                        pso = pso_mms(0, 384, nm)
                        nc.vector.tensor_copy(dst[:, 0:384], pso[:, 0:384])
                        nc.sync.dma_start(orows[:, ocol:ocol + 384],
                                          dst[:, 0:384])
                        pso2 = pso_mms(384, QBLK, nm + "b")
                        nc.scalar.copy(dst[:, 384:], pso2[:, 0:P])
                        nc.sync.dma_start(orows[:, ocol + 384:ocol + QBLK],
                                          dst[:, 384:])
                    else:
                        pso = pso_mms(0, QBLK, nm)
                        # PSUM->SBUF copies: only Act/DVE may read PSUM;
                        # alternate so neither queue serializes the WARs
                        if hbi % 2 == 0:
                            nc.vector.tensor_copy(dst, pso)
                        else:
                            nc.scalar.copy(dst, pso)
                if not last:
                    nc.sync.dma_start(
                        out[tt * P:(tt + 1) * P, half * H2:(half + 1) * H2],
                        o_t)

            # ---- PHASE B: batch-1 projections + batch-0 attention ----
            # pair a light and a heavy unit per block so Act's exp load
            # (which scales with qb) is even across phase B
            b0_units = []
            for hq in range(NH_C):
                b0_units += [(0, hq, 0), (0, hq, 3)]
            for hq in range(NH_C):
                b0_units += [(0, hq, 1), (0, hq, 2)]
            with tc.tile_pool(name="ppB", bufs=2, space="PSUM") as ppB:
                for j, tb in enumerate(range(8, NTOKBLK)):
                    proj_block(tb, ppB)
                    if tb == NTOKBLK - 1:
                        # xt buffers are done: stream Wo halves into them
                        woA_sb = singles.tile([P, NH_C, 2, H2], F8E4,
                                              tag="xtA", name="woA")
                        woB_sb = singles.tile([P, NH_C, 2, H2], F8E4,
                                              tag="xtB", name="woB")
                        nc.sync.dma_start(woA_sb, wo[:, :, :, 0:H2])
                        nc.sync.dma_start(woB_sb, wo[:, :, :, H2:HID])
                    for u in range(2):
                        b, hq, qb = b0_units[2 * j + u]
                        if tb == NTOKBLK - 1:
                            # filler: first 2 b0 p3 half-tiles (Wo halfA
                            # landed ~15us ago via the xtA alias)
                            def bfiller(u=u):
                                p3_half(2 * u, 0, ppB, woA_sb, "st")
                                p3_half(2 * u + 1, 0, ppB, woA_sb, "st")
                            attn_unit(b, hq, qb, filler=bfiller,
                                      copy_eng=nc.vector)
                        else:
                            attn_unit(b, hq, qb, copy_eng=nc.vector)

            # ---- PHASE C: batch-1 attention + output projection ----
            with tc.tile_pool(name="ps_o", bufs=2, space="PSUM") as ps_o:

                def wo_half_of(half):
                    return woA_sb if half == 0 else woB_sb

                # filler queue: (tt, half) pairs; b0's tt0-3 half0 done in B
                queue = [(tt, h) for tt in range(16) for h in range(2)
                         if not (h == 0 and tt < 4)]
                # pair light+heavy units; heavy (high-qb) units carry more
                # filler so PE outlasts Act's exp burst
                c_units = []
                for hq in range(NH_C):
                    c_units += [(hq, 3), (hq, 0)]
                for hq in range(NH_C):
                    c_units += [(hq, 2), (hq, 1)]
                done = {qb: 0 for qb in range(NQB)}
                for hq, qb in c_units:
                    nfill = {0: 2, 1: 2, 2: 2, 3: 3}[qb]

                    def filler(nfill=nfill):
                        for _ in range(nfill):
                            if queue:
                                tt, h = queue.pop(0)
                                p3_half(tt, h, ps_o, wo_half_of(h), "o")
                    attn_unit(1, hq, qb, filler=filler,
                              copy_eng=nc.vector)
                    done[qb] += 1
                    if done[qb] == NH_C:
                        # this b1 group's p3 becomes ready
                        queue.extend((16 + qb * 4 + i, h) for i in range(4)
                                     for h in range(2))
                # tail: remaining p3
                while queue:
                    tt, h = queue.pop(0)
                    last = not queue
                    p3_half(tt, h, ps_o, wo_half_of(h), "o",
                            tail=True, last=last, pool2=ps_st)

    return nc


def _split8(a):
    """f32 array -> (hi, lo) e4m3 arrays with hi+lo ~ a."""
    E4 = ml_dtypes.float8_e4m3
    hi = a.astype(E4)
    lo = (a - hi.astype(np.float32)).astype(E4)
    return hi, lo


def _prep_inputs(hidden_states, position_ids, Wq, Wk, Wv, Wo):
    bf16 = ml_dtypes.bfloat16
    E4 = ml_dtypes.float8_e4m3
    hs = np.asarray(hidden_states, np.float32).reshape(T, HID)
    x4 = np.ascontiguousarray(hs.T) * SX                     # [HID, T]
    xhi, xlo = _split8(x4)
    # block-major layout [p, tb, ho, 2, t] so every DMA slice is contiguous
    xt8 = np.empty((P, NTOKBLK, HO, 2, TOKBLK), E4)
    xt8[:, :, :, 0, :] = (xhi.reshape(HO, P, NTOKBLK, TOKBLK)
                          .transpose(1, 2, 0, 3))
    xt8[:, :, :, 1, :] = (xlo.reshape(HO, P, NTOKBLK, TOKBLK)
                          .transpose(1, 2, 0, 3))

    pos = np.asarray(position_ids, np.int64).reshape(B, S)
    inv_freq = 1.0 / (ROPE_BASE ** (np.arange(0, D, 2, dtype=np.float32) / D))
    ang = pos.astype(np.float32)[:, :, None] * inv_freq[None, None, :]
    cs_scale = 1.0 / (SX * SW)
    cosT = np.ascontiguousarray(
        np.cos(ang).reshape(T, D // 2).T * cs_scale).astype(bf16)
    sinT = np.ascontiguousarray(
        np.sin(ang).reshape(T, D // 2).T * cs_scale).astype(bf16)

    Wq_s = np.asarray(Wq, np.float32) * (SCALE * SW)
    Wk_s = np.asarray(Wk, np.float32) * SW
    Wv_s = np.asarray(Wv, np.float32) * SW
    Wo_s = np.asarray(Wo, np.float32) * SW

    maskv = np.triu(np.ones((P, P), np.float32)).astype(bf16)
    ones = np.full((P, P), ONESV, np.float32).astype(bf16)
    ones8 = np.full((P, 2 * P), ONESV, np.float32).astype(
        ml_dtypes.float8_e5m2)

    shared = {"xt": xt8, "cos": cosT, "sin": sinT, "maskv": maskv,
              "ones": ones, "ones8": ones8}
    in_maps = []
    for c in range(NCORES):
        m = dict(shared)
        wq_c = Wq_s[:, c * DQ_C:(c + 1) * DQ_C]               # [HID, 512]
        hi, lo = _split8(wq_c)
        wq8 = np.empty((P, NH_C, HO, 2, P), E4)
        wq8[:, :, :, 0, :] = (hi.reshape(HO, P, NH_C, P)
                              .transpose(1, 2, 0, 3))
        wq8[:, :, :, 1, :] = (lo.reshape(HO, P, NH_C, P)
                              .transpose(1, 2, 0, 3))
        m["wq"] = wq8
        for nme, Wfull in (("wk", Wk_s), ("wv", Wv_s)):
            w_c = Wfull[:, c * D:(c + 1) * D]                 # [HID, 128]
            hi, lo = _split8(w_c)
            w8 = np.empty((P, HO, 2, P), E4)
            w8[:, :, 0, :] = hi.reshape(HO, P, P).transpose(1, 0, 2)
            w8[:, :, 1, :] = lo.reshape(HO, P, P).transpose(1, 0, 2)
            m[nme] = w8
        wo_c = Wo_s[c * DQ_C:(c + 1) * DQ_C, :]               # [512, HID]
        hi, lo = _split8(wo_c)
        wo8 = np.empty((P, NH_C, 2, HID), E4)
        wo8[:, :, 0, :] = hi.reshape(NH_C, P, HID).transpose(1, 0, 2)
        wo8[:, :, 1, :] = lo.reshape(NH_C, P, HID).transpose(1, 0, 2)
        m["wo"] = wo8
        in_maps.append(m)
    return in_maps


_LAST_EXEC_NS = None


def legalize_sync_waits(js: bytes) -> bytes:
    """This walrus accepts at most one embedded sync wait per instruction.
    Hoist extra waits (and extra updates) onto standalone EventSemaphore
    instructions inserted just before (waits) / after (updates) on the same
    engine stream."""
    import json
    d = json.loads(js)
    n_new = 0
    for fn in d["functions"]:
        for blk in fn["blocks"]:
            out = []
            for inst in blk["instructions"]:
                si = inst.get("sync_info")
                waits = (si or {}).get("on_wait") or []
                updates = (si or {}).get("on_update") or []
                if len(waits) > 1:
                    for w in waits[:-1]:
                        n_new += 1
                        out.append({
                            "debug": inst.get("debug", 0),
                            "engine": inst["engine"],
                            "ins": [], "outs": [],
                            "name": f"{inst['name']}-hw{n_new}",
                            "opcode": "EventSemaphore",
                            "sync_info": {"on_wait": [w], "on_update": []},
                        })
                    si["on_wait"] = [waits[-1]]
                out.append(inst)
                if len(updates) > 1:
                    for u in updates[1:]:
                        n_new += 1
                        out.append({
                            "debug": inst.get("debug", 0),
                            "engine": inst["engine"],
                            "ins": [], "outs": [],
                            "name": f"{inst['name']}-hu{n_new}",
                            "opcode": "EventSemaphore",
                            "sync_info": {"on_wait": [], "on_update": [u]},
                        })
                    si["on_update"] = [updates[0]]
            blk["instructions"] = out
    return json.dumps(d).encode()


def _run(in_maps, trace=False):
    global _LAST_EXEC_NS
    nc = build_bass()
    legalized = legalize_sync_waits(nc.to_json_bytes())
    nc.to_json_bytes = lambda: legalized
    res = run_bass_kernel_spmd(nc, in_maps, core_ids=list(range(NCORES)),
                               trace=trace)
    _LAST_EXEC_NS = res.exec_time_ns
    return res.results


def kernel(hidden_states, position_ids, Wq, Wk, Wv, Wo):
    import os
    in_maps = _prep_inputs(hidden_states, position_ids, Wq, Wk, Wv, Wo)
    trace = bool(int(os.environ.get("KERNEL_TRACE", "0")))
    results = _run(in_maps, trace=trace)
    total = np.zeros((T, HID), np.float64)
    for r in results:
        total += np.asarray(r["out"]).astype(np.float64)
    total /= OUT_DIV
    return total.astype(np.float32).reshape(B, S, HID)
